# revision 18
# baseline (speedup 1.0000x reference)
"""Bayesian block-sparse linear layer (gnn message passing) on 8 Trainium2 cores.

out = segment_sum_e( v[e].T @ x_block[col_g[e]] ) + bias,
v[e] = eps_w[e] * exp(weight_log_var[e]) + weight_mean[e]   (32x32 blocks)

Measured on TRN2: the PE is instruction-issue bound at ~33-47ns per
(LDWEIGHTS+MATMUL) pair regardless of the moving free dim N, and 32x32
sub-array matmuls (tile_position) run concurrently as long as no two
sub-arrays write the same (PSUM bank, partition range).  So the design
maximizes work per matmul instruction: N=512 batch columns per matmul.

Sharding: output blocks are split across 4 distinct specialized programs
(one per row-group of the graph), each run on 2 cores that hold the two
512-column batch halves; the 4 programs execute concurrently on disjoint
core pairs.  Within a program:
  * all of x (one batch half) lives in SBUF, block c at partition group
    c%4, columns 512*(c//4); x streams in 8 column chunks and the first
    rounds' matmuls are sorted by x column so compute starts immediately;
  * per-edge weights are sampled on device (ACT exp + 2 DVE ops) from
    host-packed per-round layouts and consumed as matmul lhsT;
  * 16 rounds of 4 output blocks; block s accumulates in PSUM bank g for
    PE row-group g (4 private copies -> concurrent sub-array matmuls
    never touch the same bank+partitions, which hard-crashes the device);
    consecutive rounds use the two PSUM bank halves alternately so round
    r+1 matmuls overlap round r evacuation;
  * evacuation sums the 4 copies and adds the sampled bias.
"""

import os

import numpy as np

# problem dims (hardcoded per spec)
G1 = 256
G2 = 256
A1 = 32
A2 = 32
B = 1024
NCORES = 8

NPROG = 4
CPP = 2            # cores per program (batch halves)
NW = B // CPP      # 512 batch columns per core
NROUNDS = 16       # per program
BPR = 4            # blocks per round
BPP = G2 // NPROG  # blocks per program
NSORT = 6          # leading rounds issued in x-column order
NXCHUNK = 8

LAST_PROFILE = None
_prog_cache = {}


def _dt_mode():
    return os.environ.get("BSL_DTYPE", "bf16")


# ---------------------------------------------------------------- host plan

def _plan(row_g, col_g):
    """Specialize schedules to the graph: 4 programs x 16 rounds x 4 blocks."""
    E = len(row_g)
    blk = [[[] for _ in range(4)] for _ in range(G2)]
    for e in range(E):
        blk[int(row_g[e])][int(col_g[e]) % 4].append(e)
    cnts = np.array([[len(blk[q][g]) for g in range(4)] for q in range(G2)])

    # blocks -> programs, balancing total edge count
    order = np.argsort(-cnts.sum(1), kind="stable")
    progs = [[] for _ in range(NPROG)]
    ptot = np.zeros(NPROG, np.int64)
    for q in order:
        cand = [p for p in range(NPROG) if len(progs[p]) < BPP]
        p = min(cand, key=lambda p: ptot[p])
        progs[p].append(int(q))
        ptot[p] += cnts[q].sum()

    plans = []
    for p in range(NPROG):
        # blocks -> rounds, balancing per-partition-group load
        rounds = [[] for _ in range(NROUNDS)]
        load = np.zeros((NROUNDS, 4), np.int64)
        for q in sorted(progs[p], key=lambda q: -cnts[q].sum()):
            best, bkey = None, None
            for r in range(NROUNDS):
                if len(rounds[r]) >= BPR:
                    continue
                nl = load[r] + cnts[q]
                key = (int(nl.max()), int(nl.sum()))
                if best is None or key < bkey:
                    best, bkey = r, key
            rounds[best].append(q)
            load[best] += cnts[q]

        blkmap = {}
        for r in range(NROUNDS):
            for idx, q in enumerate(rounds[r]):
                blkmap[q] = (r, idx)

        sched = []
        for r in range(NROUNDS):
            glists = [[] for _ in range(4)]
            for q in rounds[r]:
                for g in range(4):
                    lst = blk[q][g]
                    if lst:
                        for e in lst:
                            glists[g].append((e, q))
                    else:
                        # region never written otherwise -> dummy zero mm
                        glists[g].append((E, q))
            if r < NSORT:
                # leading rounds: x-column order so matmuls start while
                # the chunked x DMA is still streaming in
                for g in range(4):
                    glists[g].sort(key=lambda eq: (
                        (int(col_g[eq[0]]) // 4) if eq[0] < E else 0))
            else:
                # slot round-robin: consecutive same-group matmuls hit
                # different PE sub-arrays
                for g in range(4):
                    byslot = [[] for _ in range(BPR)]
                    for e, q in glists[g]:
                        byslot[blkmap[q][1]].append((e, q))
                    inter = []
                    i = 0
                    while any(byslot):
                        sl = byslot[i % BPR]
                        if sl:
                            inter.append(sl.pop(0))
                        i += 1
                    glists[g] = inter
            L = max(len(x) for x in glists)
            q0 = rounds[r][0]
            for g in range(4):
                while len(glists[g]) < L:
                    glists[g].append((E, q0))
            # start/stop per (row-group g, block q): each (bank, slot)
            # region belongs to exactly one block copy, and only PE tile
            # (g, s) writes it -> the clear/accumulate order is the PE's
            # own FIFO, race-free.
            first_pos, last_pos = {}, {}
            for pp in range(L):
                for g in range(4):
                    e, q = glists[g][pp]
                    if (g, q) not in first_pos:
                        first_pos[(g, q)] = pp
                    last_pos[(g, q)] = pp
            entries = [[None] * L for _ in range(4)]
            widx = np.full((4, L), E, np.int64)
            for pp in range(L):
                for g in range(4):
                    e, q = glists[g][pp]
                    s = blkmap[q][1]
                    xcol = (int(col_g[e]) // 4) if e < E else 0
                    entries[g][pp] = (s, xcol,
                                      first_pos[(g, q)] == pp,
                                      last_pos[(g, q)] == pp)
                    widx[g, pp] = e
            sched.append({"L": L, "entries": entries, "widx": widx})
        plans.append({"sched": sched, "rounds": rounds})
    return plans


# ---------------------------------------------------------------- host pack

def _pack_weights(w, sched, np_dt):
    w = np.asarray(w, np.float32).reshape(-1, A1, A2)
    wext = np.concatenate([w, np.zeros((1, A1, A2), np.float32)], 0)
    outs = []
    for sc in sched:
        t = wext[sc["widx"]]                       # [4, L, 32, 32]
        t = t.transpose(0, 2, 1, 3).reshape(128, 32 * sc["L"])
        outs.append(np.ascontiguousarray(t.astype(np_dt)))
    return outs


def _pack_x(xk, np_dt):  # xk [8192, NW]
    t = xk.reshape(64, 4, 32, NW).transpose(1, 2, 0, 3)
    return np.ascontiguousarray(t.reshape(128, 64 * NW).astype(np_dt))


def _pack_bias(vec, rounds):  # vec [8192] fp32 -> [128, NROUNDS]
    out = np.zeros((NROUNDS, 128), np.float32)
    for r in range(NROUNDS):
        for idx, q in enumerate(rounds[r]):
            out[r, 32 * idx:32 * idx + 32] = vec[32 * q:32 * q + 32]
    return np.ascontiguousarray(out.transpose(1, 0))


def _unpack_out(op, rounds):  # op [NROUNDS, 128, NW] fp32 -> [G2, 32, NW]
    res = np.zeros((G2, 32, NW), np.float32)
    t = op.reshape(NROUNDS, 4, 32, NW)
    for r in range(NROUNDS):
        for idx, q in enumerate(rounds[r]):
            res[q] = t[r, idx]
    return res


# ---------------------------------------------------------------- program

def _build(sched, dt_w, pidx):
    import concourse.bacc as bacc
    import concourse.mybir as mybir
    import concourse.tile as tile_mod

    nc = bacc.Bacc("TRN2", target_bir_lowering=False, debug=False,
                   num_devices=CPP)
    f32 = mybir.dt.float32
    AF = mybir.ActivationFunctionType
    ADD = mybir.AluOpType.add
    MUL = mybir.AluOpType.mult

    x_d = nc.dram_tensor("x_packed", [128, 64 * NW], dt_w,
                         kind="ExternalInput")
    wm_d = [nc.dram_tensor(f"wm_{r}", [128, 32 * sched[r]["L"]], dt_w,
                           kind="ExternalInput") for r in range(NROUNDS)]
    wl_d = [nc.dram_tensor(f"wl_{r}", [128, 32 * sched[r]["L"]], dt_w,
                           kind="ExternalInput") for r in range(NROUNDS)]
    we_d = [nc.dram_tensor(f"we_{r}", [128, 32 * sched[r]["L"]], dt_w,
                           kind="ExternalInput") for r in range(NROUNDS)]
    bm_d = nc.dram_tensor("bm_packed", [128, NROUNDS], f32,
                          kind="ExternalInput")
    bl_d = nc.dram_tensor("bl_packed", [128, NROUNDS], f32,
                          kind="ExternalInput")
    be_d = nc.dram_tensor("be_packed", [128, NROUNDS], f32,
                          kind="ExternalInput")
    out_d = nc.dram_tensor("out_packed", [NROUNDS, 128, NW], f32,
                           kind="ExternalOutput")

    with tile_mod.TileContext(nc) as tc:
        with tc.tile_pool(name="xp", bufs=1) as xp, \
             tc.tile_pool(name="bp", bufs=1) as bp, \
             tc.tile_pool(name="wp", bufs=6) as wp, \
             tc.tile_pool(name="pp", bufs=8, space="PSUM") as pp, \
             tc.tile_pool(name="ep", bufs=4) as ep, \
             tc.tile_pool(name="opool", bufs=3) as opool:

            # x streams in column chunks interleaved with the leading
            # rounds' weight DMAs: arrival order matches consumption order
            xt = xp.tile([128, 64 * NW], dt_w, name="xt")
            XC = (64 * NW) // NXCHUNK
            wtiles = {}

            def w_dmas(r):
                L = sched[r]["L"]
                W = 32 * L
                wlt = wp.tile([128, W], dt_w, tag="wl", name=f"wl_t{r}")
                nc.sync.dma_start(wlt[:, :], wl_d[r].ap())
                wet = wp.tile([128, W], dt_w, tag="we", name=f"we_t{r}")
                nc.sync.dma_start(wet[:, :], we_d[r].ap())
                wmt = wp.tile([128, W], dt_w, tag="wm", name=f"wm_t{r}")
                nc.sync.dma_start(wmt[:, :], wm_d[r].ap())
                wtiles[r] = (wlt, wet, wmt)

            for c in range(NXCHUNK):
                nc.sync.dma_start(xt[:, c * XC:(c + 1) * XC],
                                  x_d.ap()[:, c * XC:(c + 1) * XC])
                if c < NSORT:
                    w_dmas(c)

            bmt = bp.tile([128, NROUNDS], f32, name="bmt")
            nc.sync.dma_start(bmt[:, :], bm_d.ap())
            blt = bp.tile([128, NROUNDS], f32, name="blt")
            nc.sync.dma_start(blt[:, :], bl_d.ap())
            bet = bp.tile([128, NROUNDS], f32, name="bet")
            nc.sync.dma_start(bet[:, :], be_d.ap())
            bias = bp.tile([128, NROUNDS], f32, name="bias")
            nc.scalar.activation(bias[:, :], blt[:, :], AF.Exp)
            nc.vector.tensor_tensor(bias[:, :], bias[:, :], bet[:, :], MUL)
            nc.vector.tensor_tensor(bias[:, :], bias[:, :], bmt[:, :], ADD)

            for r in range(NROUNDS):
                L = sched[r]["L"]
                W = 32 * L
                if r not in wtiles:
                    w_dmas(r)
                wlt, wet, wmt = wtiles.pop(r)

                # v = eps * exp(log_var) + mean, in column halves so the
                # first matmuls can start before the full round is sampled
                half = 32 * ((L + 1) // 2)
                for c0, c1 in ((0, half), (half, W)):
                    if c0 >= c1:
                        continue
                    nc.scalar.activation(wlt[:, c0:c1], wlt[:, c0:c1], AF.Exp)
                    nc.vector.tensor_tensor(wet[:, c0:c1], wet[:, c0:c1],
                                            wlt[:, c0:c1], MUL)
                    nc.vector.tensor_tensor(wet[:, c0:c1], wet[:, c0:c1],
                                            wmt[:, c0:c1], ADD)

                # bank g is private to PE row-group g: no two sub-array
                # matmuls ever write the same (bank, partitions); pool
                # slots alternate bank halves so round r+1 overlaps r evac
                banks = [pp.tile([128, NW], f32, tag="bank",
                                 name=f"bank{r}_{b}") for b in range(4)]
                ents = sched[r]["entries"]
                for pp_i in range(L):
                    for g in range(4):
                        s, xcol, st, sp = ents[g][pp_i]
                        nc.tensor.matmul(
                            banks[g][32 * s:32 * s + 32, 0:NW],
                            lhsT=wet[32 * g:32 * g + 32,
                                     32 * pp_i:32 * pp_i + 32],
                            rhs=xt[32 * g:32 * g + 32,
                                   NW * xcol:NW * xcol + NW],
                            start=st, stop=sp, skip_group_check=True,
                            tile_position=(32 * g, 32 * s))

                # out = sum of the 4 row-group copies + bias
                # (DVE can take at most one PSUM operand per instruction)
                t1 = ep.tile([128, NW], f32, tag="eacc", name=f"t1_{r}")
                nc.scalar.activation(t1[:, :], banks[0][:, :], AF.Identity,
                                     bias=0.0)
                for g in range(1, 4):
                    nc.vector.tensor_tensor(t1[:, :], t1[:, :],
                                            banks[g][:, :], ADD)
                ot = opool.tile([128, NW], f32, tag="ot", name=f"ot{r}")
                nc.scalar.activation(ot[:, :], t1[:, :], AF.Identity,
                                     bias=bias[:, r:r + 1])
                nc.sync.dma_start(out_d.ap()[r, :, :], ot[:, :])

    nc.compile()
    return nc


# ---------------------------------------------------------------- runner

def _dispatch_pjrt(nc, in_maps, devices):
    """Async-dispatch one Bass program on a specific device subset.

    Clone of concourse.bass2jax.run_bass_via_pjrt's multi-core branch with
    an explicit device list; returns un-materialized jax arrays so several
    programs can execute concurrently on disjoint core pairs.
    """
    import jax
    import concourse.mybir as mybir
    from concourse import bass2jax
    from jax.sharding import Mesh, PartitionSpec
    from jax.experimental.shard_map import shard_map

    bass2jax.install_neuronx_cc_hook()
    assert nc.dbg_addr is None
    partition_name = (nc.partition_id_tensor.name
                      if nc.partition_id_tensor else None)

    in_names, out_names, out_avals, zero_outs = [], [], [], []
    for alloc in nc.m.functions[0].allocations:
        if not isinstance(alloc, mybir.MemoryLocationSet):
            continue
        name = alloc.memorylocations[0].name
        if alloc.kind == "ExternalInput":
            if name != partition_name:
                in_names.append(name)
        elif alloc.kind == "ExternalOutput":
            out_names.append(name)
            shape = tuple(alloc.tensor_shape)
            dtype = mybir.dt.np(alloc.dtype)
            out_avals.append(jax.core.ShapedArray(shape, dtype))
            zero_outs.append(np.zeros(shape, dtype))
    n_params = len(in_names)
    n_outs = len(out_avals)
    in_names = in_names + out_names
    if partition_name is not None:
        in_names.append(partition_name)
    donate = tuple(range(n_params, n_params + n_outs))

    def _body(*args):
        operands = list(args)
        if partition_name is not None:
            operands.append(bass2jax.partition_id_tensor())
        outs = bass2jax._bass_exec_p.bind(
            *operands,
            out_avals=tuple(out_avals),
            in_names=tuple(in_names),
            out_names=tuple(out_names),
            lowering_input_output_aliases=(),
            sim_require_finite=True,
            sim_require_nnan=True,
            nc=nc,
        )
        return tuple(outs)

    n_cores = len(devices)
    mesh = Mesh(np.asarray(devices), ("core",))
    sharded = jax.jit(
        shard_map(_body, mesh=mesh,
                  in_specs=(PartitionSpec("core"),) * (n_params + n_outs),
                  out_specs=(PartitionSpec("core"),) * n_outs,
                  check_rep=False),
        donate_argnums=donate, keep_unused=True)
    per_core = [[np.asarray(m[name]) for name in in_names[:n_params]]
                for m in in_maps]
    concat_in = [np.concatenate([per_core[c][i] for c in range(n_cores)], 0)
                 for i in range(n_params)]
    concat_zeros = [np.zeros((n_cores * z.shape[0], *z.shape[1:]), z.dtype)
                    for z in zero_outs]
    out_arrs = sharded(*concat_in, *concat_zeros)
    return out_arrs, out_names, out_avals, n_cores


def _run_programs(ncs, maps_list, trace):
    """Run the programs concurrently on disjoint device pairs."""
    import jax
    devices = jax.devices()

    def dispatch_all():
        handles = []
        for q, (nc, maps) in enumerate(zip(ncs, maps_list)):
            devs = devices[CPP * q:CPP * (q + 1)]
            handles.append(_dispatch_pjrt(nc, maps, devs))
        return handles

    prof = {"exec_time_ns": None, "mean_exec_time_ns": None,
            "max_exec_time_core_id": None, "trace": None, "insts": None,
            "per_core_ns": None}

    if not trace:
        handles = dispatch_all()
    else:
        handles = None
        try:
            import glob as globmod
            import re
            import shutil
            import tempfile
            import time as time_mod
            from antenv.axon_hooks import get_axon_ntff_profile_hook
            hook = get_axon_ntff_profile_hook()
            neff_dir = tempfile.mkdtemp()
            with hook(neff_dir, list(range(NCORES))):
                t0 = time_mod.time()
                handles = dispatch_all()
                for out_arrs, _, _, _ in handles:
                    for a in out_arrs:
                        a.block_until_ready()
                wall_s = time_mod.time() - t0
            prof["wall_s"] = wall_s
            ntffs = globmod.glob(os.path.join(neff_dir, "*_body*.ntff"))
            if ntffs:
                import gauge.profiler
                from concourse._compat import FishPath
                # group by executable (one per program, in dispatch order)
                exids = sorted({re.search(r"executable(\d+)", f).group(1)
                                for f in ntffs})
                per_core = {}
                insts = None
                tracep = None
                best = -1
                for qi, exid in enumerate(exids):
                    sub = os.path.join(neff_dir, f"ex{exid}")
                    os.makedirs(sub, exist_ok=True)
                    for f in os.listdir(neff_dir):
                        if f"executable{exid}" in f:
                            shutil.move(os.path.join(neff_dir, f),
                                        os.path.join(sub, f))
                    profile = gauge.profiler.Profile(
                        profile_path=FishPath(sub),
                        kernel_dev_mode=True, profile_on_exit=False,
                        bass_kernel=ncs[min(qi, len(ncs) - 1)].m,
                        offline_processing=True, fname="*_body*")
                    results = profile.to_perfetto(model_index=(0, 1))
                    for ci, pr in enumerate(results or []):
                        per_core[(qi, ci)] = pr.exec_time_ns
                        if (pr.exec_time_ns or 0) > best:
                            best = pr.exec_time_ns or 0
                            insts, tracep = pr.insts, pr.trace_path
                vals = [v for v in per_core.values() if v]
                if vals:
                    prof.update(
                        exec_time_ns=max(vals),
                        mean_exec_time_ns=sum(vals) / len(vals),
                        max_exec_time_core_id=str(max(
                            per_core, key=lambda c: per_core[c] or 0)),
                        per_core_ns={str(k): v for k, v in per_core.items()},
                        insts=insts, trace=tracep)
        except Exception as exc:  # profiling must never break the run
            print(f"[kernel] trace failed: {type(exc).__name__}: {exc}")
            if handles is None:
                handles = dispatch_all()

    results = []
    for out_arrs, out_names, out_avals, n_cores in handles:
        cores = []
        for c in range(n_cores):
            cores.append({
                name: np.asarray(out_arrs[i]).reshape(
                    n_cores, *out_avals[i].shape)[c]
                for i, name in enumerate(out_names)})
        results.append(cores)
    return results, prof


# ---------------------------------------------------------------- entry

def _get_programs(row_g, col_g):
    import concourse.mybir as mybir
    import ml_dtypes
    mode = _dt_mode()
    key = (row_g.tobytes(), col_g.tobytes(), mode)
    if key not in _prog_cache:
        plans = _plan(row_g, col_g)
        dt_w = mybir.dt.float32 if mode == "fp32" else mybir.dt.bfloat16
        np_dt = np.float32 if mode == "fp32" else ml_dtypes.bfloat16
        ncs = [_build(plans[p]["sched"], dt_w, p) for p in range(NPROG)]
        _prog_cache[key] = (ncs, plans, np_dt)
    return _prog_cache[key]


def kernel(**inputs):
    global LAST_PROFILE

    row_g = np.asarray(inputs["row_g"])
    col_g = np.asarray(inputs["col_g"])
    ncs, plans, np_dt = _get_programs(row_g, col_g)

    x = np.asarray(inputs["x"], np.float32)
    xpk = [_pack_x(np.ascontiguousarray(x[:, h * NW:(h + 1) * NW]), np_dt)
           for h in range(CPP)]
    bm = np.asarray(inputs["b_mean"], np.float32)
    bl = np.asarray(inputs["b_log_var"], np.float32)
    be = np.asarray(inputs["eps_b"], np.float32)

    maps_list = []
    for p in range(NPROG):
        sched, rounds = plans[p]["sched"], plans[p]["rounds"]
        wm = _pack_weights(inputs["weight_mean"], sched, np_dt)
        wl = _pack_weights(inputs["weight_log_var"], sched, np_dt)
        we = _pack_weights(inputs["eps_w"], sched, np_dt)
        shared = {f"wm_{r}": wm[r] for r in range(NROUNDS)}
        shared.update({f"wl_{r}": wl[r] for r in range(NROUNDS)})
        shared.update({f"we_{r}": we[r] for r in range(NROUNDS)})
        shared["bm_packed"] = _pack_bias(bm, rounds)
        shared["bl_packed"] = _pack_bias(bl, rounds)
        shared["be_packed"] = _pack_bias(be, rounds)
        maps_list.append([{**shared, "x_packed": xpk[h]} for h in range(CPP)])

    trace = os.environ.get("BSL_TRACE", "0") == "1"
    results, prof = _run_programs(ncs, maps_list, trace)
    LAST_PROFILE = prof

    out = np.zeros((G2 * A2, B), np.float32)
    for p in range(NPROG):
        rounds = plans[p]["rounds"]
        mask = np.zeros(G2, bool)
        for r in range(NROUNDS):
            for q in rounds[r]:
                mask[q] = True
        for h in range(CPP):
            res = _unpack_out(results[p][h]["out_packed"], rounds)
            out.reshape(G2, 32, B)[mask, :, h * NW:(h + 1) * NW] = res[mask]
    return out, np.float32(0.0)
